# revision 37
# baseline (speedup 1.0000x reference)
"""Trainium2 Bass kernel for DynamicDirectionalConv (v5).

Math (per batch b):
  x_low = einsum('chw,mc->mhw', x, w_reduce)                 # 1x1 reduce C=256->16
  w_h   = cos(angle)^2
  out_low = w_h * (x_low (*) BASE_H) + (1-w_h) * (x_low (*) BASE_V)
  out   = einsum('mhw,cm->chw', out_low, w_expand)           # 1x1 expand 16->256

Both base kernels are separable rank-1 7x7 gaussians with reflect
padding, and the per-pixel blend factors out of the tap sum.

v5 structure -- every linear stage runs on the PE:
  * s1 reduce: per h-row, two 16-col matmuls (c-halves) with the x
    tile stationary -> X3[w, (h m)] (cheap: ldweights pipelines).
  * W-conv + transpose fused: psW[(h8,m16), w'] = X3_block.T @ TwT
    (banded reflect matrix as the moving operand, X3 block
    stationary). One matmul per 8-row block per path.
  * H-conv as a BLOCK-TRIDIAGONAL matmul in the transposed layout:
    h lives in partitions (8 rows per block), so the 7-tap reflect
    conv over h is out[q] = sum_d L[d].T-style products with
    d in {-1,0,+1}; L matrices are shift-invariant except the first /
    last block (reflect folds). Stationaries are reused across blocks
    (ordered by d), accumulation in PSUM.
  * blend in the [(h m), w] layout against a host-prepared whT map.
  * expand 16->256 with the zero-masked replicated weight trick,
    per-(hlp,cc,j2) PSUM tiles, drains alternating DVE/Act,
    2 output DMAs per chunk (contiguous 2KB runs in DRAM).
  * ~8 zero matmuls at t~7.5us (gated only by a tiny const DMA) keep
    the PE busy during the input-DMA head so it ramps to full p-state
    before real work arrives.

Sharding: data-parallel over batch, 1 batch per NeuronCore (B=8).
"""

import math

import numpy as np

import concourse.bass as bass
import concourse.tile as tile
from concourse import mybir
import bass_rust
from concourse.bass_utils import run_bass_kernel_spmd

B, C, H, W, MID = 8, 256, 128, 128, 16
K, PAD = 7, 3
F32 = mybir.dt.float32
BF16 = mybir.dt.bfloat16

NCH = 32  # output rows per chunk
NCHUNK = H // NCH
NB = H // 8  # 16 8-row blocks


# ----------------------------------------------------------------- host consts
def _refl(t, n):
    if t < 0:
        return -t
    if t > n - 1:
        return 2 * (n - 1) - t
    return t


def _banded_reflect(g, n):
    """T[out, in]: out[o] = sum_t g[t] * x[refl(o + t - PAD)]."""
    T = np.zeros((n, n), dtype=np.float64)
    for o in range(n):
        for t in range(K):
            T[o, _refl(o + t - PAD, n)] += g[t]
    return T


def _host_consts():
    ax = np.linspace(-(K // 2), K // 2, K, dtype=np.float64)
    e_w = np.exp(-(ax**2) / (2 * 2.5**2))  # wide gaussian (sigma_h)
    e_n = np.exp(-(ax**2) / (2 * 1.0**2))  # narrow gaussian (sigma_v)
    s_h = float(np.outer(e_w, e_n).sum()) + 1e-8
    s_v = float(np.outer(e_n, e_w).sum()) + 1e-8
    gh = [e_w, e_n]  # h-axis taps for paths A, B (unnormalized)
    gw = [e_n / s_h, e_w / s_v]  # w-axis taps carry the normalization

    Th = [_banded_reflect(g, H) for g in gh]  # [h_out, h_in] per path
    TwT = [np.ascontiguousarray(_banded_reflect(g, W).T) for g in gw]

    # block-tridiagonal factorization of Th into (delta, variant) L mats:
    # L[p=(hl,m), p'=(hl',m')] = Th[8*q_out+hl', 8*q_in+hl] * (m==m')
    # variants: 0:interior d=-1, 1:interior d=0, 2:interior d=+1,
    #           3:d=0 for block 0 (top folds), 4:d=0 for block 15
    Ls = []
    for Tp in Th:
        mats = []
        for (qo, qi) in ((1, 0), (1, 1), (1, 2), (0, 0), (NB - 1, NB - 1)):
            Lm = np.zeros((128, 128), np.float64)
            for hl in range(8):
                for hlp in range(8):
                    v = Tp[8 * qo + hlp, 8 * qi + hl]
                    if v != 0.0:
                        for m in range(MID):
                            Lm[hl * MID + m, hlp * MID + m] = v
            mats.append(Lm)
        Ls.append(mats)
        # verify: assembled block-tridiag reproduces Th exactly
        full = np.zeros((H, H))
        for qo in range(NB):
            for qi in range(NB):
                d = qi - qo
                if abs(d) > 1:
                    assert np.allclose(Tp[8*qo:8*qo+8, 8*qi:8*qi+8], 0)
                    continue
                if d == 0:
                    Lm = mats[3] if qo == 0 else (mats[4] if qo == NB - 1 else mats[1])
                else:
                    Lm = mats[d + 1]
                blk = np.zeros((8, 8))
                for hl in range(8):
                    for hlp in range(8):
                        blk[hlp, hl] = Lm[hl * MID, hlp * MID]
                full[8*qo:8*qo+8, 8*qi:8*qi+8] = blk
        assert np.allclose(full, Tp), "block-tridiag mismatch"
    return gh, Th, TwT, Ls


GH, TH, TWT, LS = _host_consts()

# const layouts
NC_EARLY = 32 + 512  # wrT halves [128,16]x2 + zero row region

OFF_TWTA = 0
OFF_TWTB = 128
OFF_L = 256                        # 3 "paths" (B, A, -B) x 5 variants x 128
OFF_WET = OFF_L + 15 * 128         # 2176
OFF_WHT = OFF_WET + 4 * C          # 3200
NC_LATE = OFF_WHT + H * MID        # 5248


def _build_const_early(w_reduce):
    ce = np.zeros((128, NC_EARLY), dtype=np.float64)
    wrT = w_reduce.T.astype(np.float64)  # [C, MID]
    ce[:, 0:MID] = wrT[0:128]
    ce[:, MID:2 * MID] = wrT[128:256]
    return ce


def _build_const_late(w_expand):
    cl = np.zeros((128, NC_LATE), dtype=np.float64)
    cl[:, OFF_TWTA:OFF_TWTA + 128] = TWT[0]
    cl[:, OFF_TWTB:OFF_TWTB + 128] = TWT[1]
    # L sets: 0 -> path B, 1 -> path A, 2 -> negated path B (for the
    # PE-computed difference path D = A - B)
    for p, mats in enumerate((LS[1], LS[0], [-m for m in LS[1]])):
        for v in range(5):
            o = OFF_L + (p * 5 + v) * 128
            cl[:, o:o + 128] = mats[v]
    weT = w_expand.T.astype(np.float64)  # [MID, C]
    wet = np.zeros((128, 4 * C), np.float64)
    for p in range(128):
        v = (p // 16) % 4
        wet[p, v * C:(v + 1) * C] = weT[p % 16]
    cl[:, OFF_WET:OFF_WET + 4 * C] = wet
    return cl


def _whT(angle_map_b):
    wh = np.cos(np.asarray(angle_map_b, np.float64)) ** 2  # [H, W]
    hl = np.arange(128) // MID
    kb = np.arange(NB)
    return wh[(8 * kb[None, :] + hl[:, None])].reshape(128, H * MID)


# ----------------------------------------------------------------- bass module
def build_nc(split_multiwaits=True):
    nc = bass.Bass()

    x_in = nc.dram_tensor("x", [C, H, W], BF16, kind="ExternalInput")
    ce_in = nc.dram_tensor("c_early", [128, NC_EARLY], BF16, kind="ExternalInput")
    cl_in = nc.dram_tensor("c_late", [128, NC_LATE], BF16, kind="ExternalInput")
    out_dram = nc.dram_tensor("out", [C, H, W], BF16, kind="ExternalOutput")

    from contextlib import ExitStack

    with tile.TileContext(nc) as tc, ExitStack() as es:
        consts = es.enter_context(tc.tile_pool(name="consts", bufs=1))
        xpool = es.enter_context(tc.tile_pool(name="xpool", bufs=1))
        x3pool = es.enter_context(tc.tile_pool(name="x3", bufs=1))
        uwpool = es.enter_context(tc.tile_pool(name="uw", bufs=1))
        zpool = es.enter_context(tc.tile_pool(name="z", bufs=2))
        olppool = es.enter_context(tc.tile_pool(name="olp", bufs=3))
        opool = es.enter_context(tc.tile_pool(name="ostage", bufs=2))
        ps1pool = es.enter_context(tc.tile_pool(name="ps1", bufs=1, space="PSUM"))
        pswpool = es.enter_context(tc.tile_pool(name="psw", bufs=1, space="PSUM"))
        # psh (H-conv out) and pso (expand out) share one rotating pool of
        # 2-bank tiles: more runway between a matmul and the drain that
        # frees its slot, so the PE rarely stalls on slot rotation
        psopool = es.enter_context(tc.tile_pool(name="pso", bufs=3, space="PSUM"))
        pshpool = psopool

        cearly = consts.tile([128, NC_EARLY], BF16)
        nc.sync.dma_start(out=cearly, in_=ce_in[:])

        xt = [[None, None] for _ in range(8)]

        def dma_x(g):
            for ch in range(2):
                t = xpool.tile([128, 16, W], BF16, tag=f"x{g}_{ch}")
                nc.sync.dma_start(
                    out=t, in_=x_in[ch * 128:(ch + 1) * 128, g * 16:(g + 1) * 16, :]
                )
                xt[g][ch] = t

        # issue order tuned to expected consumption times: conv consts
        # (TwT + L) after the first two x groups, the fat expand/blend
        # consts (wet + whT) before the first blend needs them
        clate = consts.tile([128, NC_LATE], BF16)
        for g in range(2):
            dma_x(g)
        nc.sync.dma_start(out=clate[:, 0:OFF_WET], in_=cl_in[:, 0:OFF_WET])
        for g in range(2, 4):
            dma_x(g)
        nc.sync.dma_start(out=clate[:, OFF_WET:], in_=cl_in[:, OFF_WET:])
        for g in range(4, 8):
            dma_x(g)

        WR = [cearly[:, 0:MID], cearly[:, MID:2 * MID]]
        ZROW = cearly[0:1, 32:32 + 512]
        ZCOL = cearly[0:1, 32:32 + 128]
        TWTv = [clate[:, OFF_TWTA:OFF_TWTA + 128],
                clate[:, OFF_TWTB:OFF_TWTB + 128]]
        Lv = [
            [clate[:, OFF_L + (p * 5 + v) * 128:OFF_L + (p * 5 + v + 1) * 128]
             for v in range(5)]
            for p in range(3)  # 0: B, 1: A, 2: -B
        ]
        WETv = clate[:, OFF_WET:OFF_WET + 4 * C]
        WHTv = clate[:, OFF_WHT:OFF_WHT + H * MID].rearrange(
            "p (kb w) -> p kb w", kb=NB
        )

        X3 = x3pool.tile([128, H * MID], BF16)  # [w, (h m)]
        Uw = [uwpool.tile([128, NB, W], BF16, tag=f"uw{p}", name=f"uw{p}")
              for p in range(2)]  # [(hl m), (blk, w')] per path

        ps1s, pshs, olps = {}, {}, {}

        def emit_warmup(n):
            """dummy matmuls into the ps1 slot: ramp PE during DMA head."""
            for i in range(n):
                ps1 = ps1pool.tile([128, 512], F32, tag="ps1")
                nc.tensor.matmul(
                    ps1, lhsT=ZCOL, rhs=ZROW,
                    start=True, stop=True, skip_group_check=True,
                )

        def emit_s1(k):
            """x_low for 32 h rows -> psum [w, (hl m)]."""
            ps1 = ps1pool.tile([128, 512], F32, tag="ps1")
            ps1s[k] = ps1
            for hl in range(32):
                h = NCH * k + hl
                fo = hl * MID
                nc.tensor.matmul(
                    ps1[:, fo:fo + MID], lhsT=xt[h // 16][0][:, h % 16, :],
                    rhs=WR[0], start=True, stop=False,
                )
                nc.tensor.matmul(
                    ps1[:, fo:fo + MID], lhsT=xt[h // 16][1][:, h % 16, :],
                    rhs=WR[1], start=False, stop=True,
                )

        def emit_s1drain(k):
            eng = nc.vector.tensor_copy if k % 2 == 0 else nc.scalar.copy
            kw = {"out": X3[:, NCH * MID * k:NCH * MID * (k + 1)], "in_": ps1s[k]}
            eng(**kw)

        def emit_wt(k):
            """psW[(hl m), w'] = X3_block.T @ TwT; drain to Uw. Paths
            run sequentially through a single 1-bank psum tile (the PE
            has slack in the input phase; this frees a bank for pso)."""
            for p in range(2):
                psw = pswpool.tile([128, 4, W], F32, tag="psw")
                for kb in range(4):
                    qb = 4 * k + kb
                    nc.tensor.matmul(
                        psw[:, kb],
                        lhsT=X3[:, qb * 128:(qb + 1) * 128],
                        rhs=TWTv[p], start=True, stop=True,
                    )
                if p == 0:
                    nc.vector.tensor_copy(
                        out=Uw[0][:, 4 * k:4 * k + 4, :], in_=psw
                    )
                else:
                    nc.scalar.copy(out=Uw[1][:, 4 * k:4 * k + 4, :], in_=psw)

        def emit_ht(k):
            """block-tridiagonal H-conv: psH[q] = sum_d L[d] blocks."""
            psh = pshpool.tile([128, 2, 4, W], F32, tag="pso", name=f"psh{k}")
            pshs[k] = psh
            # slot 0 accumulates path B; slot 1 accumulates the
            # difference path D = A - B (extra negated-L taps), so the
            # blend needs only 2 DVE ops and no gpsimd on the critical
            # path. Wide rhs applies the same L block-diagonally; each
            # psum region's accumulation group stays consecutive.
            passes = [(0, [(0, 1)]), (1, [(1, 0), (2, 1)])]
            for slot, sp in passes:
                mms = []
                if k == 0:
                    for s, pu in sp:
                        mms += [
                            (Lv[s][3], Uw[pu][:, 0, :]),
                            (Lv[s][2], Uw[pu][:, 1, :]),
                        ]
                    for i, (L, rhs) in enumerate(mms):
                        nc.tensor.matmul(
                            psh[:, slot, 0], lhsT=L, rhs=rhs,
                            start=(i == 0), stop=(i == len(mms) - 1),
                        )
                    mms = []
                    for s, pu in sp:
                        mms += [
                            (Lv[s][0], Uw[pu][:, 0:3, :]),
                            (Lv[s][1], Uw[pu][:, 1:4, :]),
                            (Lv[s][2], Uw[pu][:, 2:5, :]),
                        ]
                    for i, (L, rhs) in enumerate(mms):
                        nc.tensor.matmul(
                            psh[:, slot, 1:4], lhsT=L, rhs=rhs,
                            start=(i == 0), stop=(i == len(mms) - 1),
                        )
                elif k == NCHUNK - 1:
                    mms = [(Lv[s][v], Uw[pu][:, 11 + v:14 + v, :])
                           for s, pu in sp for v in range(3)]
                    for i, (L, rhs) in enumerate(mms):
                        nc.tensor.matmul(
                            psh[:, slot, 0:3], lhsT=L, rhs=rhs,
                            start=(i == 0), stop=(i == len(mms) - 1),
                        )
                    mms = []
                    for s, pu in sp:
                        mms += [
                            (Lv[s][0], Uw[pu][:, 14, :]),
                            (Lv[s][4], Uw[pu][:, 15, :]),
                        ]
                    for i, (L, rhs) in enumerate(mms):
                        nc.tensor.matmul(
                            psh[:, slot, 3], lhsT=L, rhs=rhs,
                            start=(i == 0), stop=(i == len(mms) - 1),
                        )
                else:
                    q0 = 4 * k
                    mms = []
                    for s, pu in sp:
                        mms += [
                            (Lv[s][0], Uw[pu][:, q0 - 1:q0 + 3, :]),
                            (Lv[s][1], Uw[pu][:, q0:q0 + 4, :]),
                            (Lv[s][2], Uw[pu][:, q0 + 1:q0 + 5, :]),
                        ]
                    for i, (L, rhs) in enumerate(mms):
                        nc.tensor.matmul(
                            psh[:, slot], lhsT=L, rhs=rhs,
                            start=(i == 0), stop=(i == len(mms) - 1),
                        )

        def emit_blend(k):
            """OLp = B + whT*D in [(hl m), (kb w)] layout; D = A - B was
            already formed on the PE, so this is 2 DVE ops (each with
            one PSUM operand) and nothing else on the critical path."""
            psh = pshs[k]
            p = zpool.tile([128, 4, W], BF16, tag="p")
            nc.vector.tensor_mul(
                out=p, in0=psh[:, 1], in1=WHTv[:, 4 * k:4 * k + 4, :]
            )
            OLp = olppool.tile([128, 4 * W], BF16, tag="olp")
            OLr = OLp.rearrange("p (kb w) -> p kb w", kb=4)
            nc.vector.tensor_add(out=OLr, in0=p, in1=psh[:, 0])
            olps[k] = OLp

        def emit_filler(n):
            """zero matmuls (no data deps beyond cearly): keep the PE
            busy through drain-gated or DMA-gated stretches so it holds
            full p-state."""
            for _ in range(n):
                ps1 = ps1pool.tile([128, 512], F32, tag="ps1")
                nc.tensor.matmul(
                    ps1, lhsT=ZCOL, rhs=ZROW,
                    start=True, stop=True, skip_group_check=True,
                )

        def emit_expand(k, hlps, osts, weave=None):
            """expand 16->256 for a subset of hlp values (+ drains); the
            optional `weave` callback emits other PE work between hlp
            pieces so pso-slot waits never idle the PE. j2 pairs share a
            2-bank pso tile drained by one 1024-col copy."""
            OLp = olps[k]
            for hlp in hlps:
                for cc in range(2):
                    pso = psopool.tile([128, 2, 4, W], F32, tag="pso")
                    for j2 in range(2):
                        hl = hlp * 2 + j2
                        b = 64 * (hl // 4)
                        v = hl % 4
                        nc.tensor.matmul(
                            pso[:, j2],
                            lhsT=WETv[b:b + 64,
                                      (v * 2 + cc) * 128:(v * 2 + cc + 1) * 128],
                            rhs=OLp[b:b + 64, :],
                            start=True, stop=True,
                            tile_position=(b, 0),
                        )
                    # both engines drain one j2 half each, in parallel
                    nc.vector.tensor_copy(
                        out=osts[cc][:, :, hlp * 2, :], in_=pso[:, 0]
                    )
                    nc.scalar.copy(
                        out=osts[cc][:, :, hlp * 2 + 1, :], in_=pso[:, 1]
                    )
                if weave is not None:
                    weave(hlp)

        def emit_ost(k):
            osts = []
            for cc in range(2):
                ost = opool.tile([128, 4, 8, W], BF16, tag=f"ost{cc}")
                osts.append(ost)
            return osts

        def emit_outdma(k, osts):
            for cc in range(2):
                o_r = out_dram[cc * 128:(cc + 1) * 128].rearrange(
                    "c (kb hl) w -> c kb hl w", hl=8
                )
                nc.sync.dma_start(
                    out=o_r[:, 4 * k:4 * k + 4, :, :], in_=osts[cc]
                )

        # software pipeline, ordered by expected operand readiness so the
        # in-order PE queue never head-blocks; expand matmuls are woven
        # with the next chunk's conv matmuls (or fillers) so pso-slot
        # drain waits never idle the PE and it holds full p-state
        # ready-time-driven: chunk 0's full path races ahead of the
        # input stream; the drain-paced expand stretches are woven at
        # per-hlp granularity with later chunks' s1/wt/ht matmuls so the
        # PE keeps streaming (and its p-state) while drains catch up
        emit_warmup(8)
        emit_s1(0)
        emit_s1drain(0)
        emit_wt(0)
        emit_s1(1)
        emit_s1drain(1)
        emit_wt(1)
        emit_ht(0)
        emit_blend(0)
        emit_s1(2)
        emit_s1drain(2)
        ost0 = emit_ost(0)

        def w0(h):
            if h == 0:
                emit_wt(2)
            elif h == 1:
                emit_s1(3)
                emit_s1drain(3)
            elif h == 2:
                emit_ht(1)
        emit_expand(0, range(4), ost0, weave=w0)
        emit_outdma(0, ost0)
        emit_blend(1)
        ost1 = emit_ost(1)

        def w1(h):
            if h == 0:
                emit_wt(3)
            elif h == 2:
                emit_ht(2)
        emit_expand(1, range(4), ost1, weave=w1)
        emit_outdma(1, ost1)
        emit_blend(2)
        ost2 = emit_ost(2)
        emit_expand(2, range(4), ost2,
                    weave=lambda h: emit_ht(3) if h == 1 else None)
        emit_outdma(2, ost2)
        emit_blend(3)
        ost3 = emit_ost(3)
        emit_expand(3, range(4), ost3)
        # split the final chunk's output DMA per hl-pair so transfers
        # start while the last drains are still running
        for cc in range(2):
            o_r = out_dram[cc * 128:(cc + 1) * 128].rearrange(
                "c (kb hl) w -> c kb hl w", hl=8
            )
            for hlp in range(4):
                nc.sync.dma_start(
                    out=o_r[:, 12:16, hlp * 2:hlp * 2 + 2, :],
                    in_=ost3[cc][:, :, hlp * 2:hlp * 2 + 2, :],
                )

    if split_multiwaits:
        _split_multiwaits(nc)
    return nc


def _split_multiwaits(nc):
    """Walrus in this toolchain accepts at most one sync-wait per
    instruction; hoist extras onto same-engine nops just before it."""
    n_new = 0
    for f in nc.m.functions:
        for bb in f.blocks:
            out, changed = [], False
            for ins in bb.instructions:
                si = ins.sync_info
                if si is not None and len(si.on_wait) > 1:
                    changed = True
                    waits = list(si.on_wait)
                    for w in waits[:-1]:
                        n_new += 1
                        nop = bass_rust.InstNoOp(
                            name=f"I-mwsplit-{n_new}", engine=ins.engine
                        )
                        nop.sync_info = mybir.SyncInfo(on_wait=[w], on_update=[])
                        out.append(nop)
                    ins.sync_info = mybir.SyncInfo(
                        on_wait=[waits[-1]], on_update=list(si.on_update)
                    )
                out.append(ins)
            if changed:
                bb.instructions = out
    return n_new


_NC = None


def _get_nc():
    global _NC
    if _NC is None:
        _NC = build_nc()
    return _NC


def make_in_maps(x, angle_map, w_reduce, w_expand):
    bf = mybir.dt.np(BF16)
    ce = _build_const_early(np.asarray(w_reduce, np.float64)).astype(bf)
    cl_shared = _build_const_late(np.asarray(w_expand, np.float64))
    maps = []
    for i in range(B):
        cl = cl_shared.copy()
        cl[:, OFF_WHT:OFF_WHT + H * MID] = _whT(angle_map[i])
        maps.append(
            {
                "x": np.ascontiguousarray(np.asarray(x[i])).astype(bf),
                "c_early": ce,
                "c_late": cl.astype(bf),
            }
        )
    return maps


def kernel(x, angle_map, w_reduce, w_expand):
    nc = _get_nc()
    in_maps = make_in_maps(x, angle_map, w_reduce, w_expand)
    res = run_bass_kernel_spmd(nc, in_maps, core_ids=list(range(B)))
    return np.stack([r["out"] for r in res.results]).astype(np.float32)


# revision 38
# speedup vs baseline: 1.0038x; 1.0038x over previous
"""Trainium2 Bass kernel for DynamicDirectionalConv (v5).

Math (per batch b):
  x_low = einsum('chw,mc->mhw', x, w_reduce)                 # 1x1 reduce C=256->16
  w_h   = cos(angle)^2
  out_low = w_h * (x_low (*) BASE_H) + (1-w_h) * (x_low (*) BASE_V)
  out   = einsum('mhw,cm->chw', out_low, w_expand)           # 1x1 expand 16->256

Both base kernels are separable rank-1 7x7 gaussians with reflect
padding, and the per-pixel blend factors out of the tap sum.

v5 structure -- every linear stage runs on the PE:
  * s1 reduce: per h-row, two 16-col matmuls (c-halves) with the x
    tile stationary -> X3[w, (h m)] (cheap: ldweights pipelines).
  * W-conv + transpose fused: psW[(h8,m16), w'] = X3_block.T @ TwT
    (banded reflect matrix as the moving operand, X3 block
    stationary). One matmul per 8-row block per path.
  * H-conv as a BLOCK-TRIDIAGONAL matmul in the transposed layout:
    h lives in partitions (8 rows per block), so the 7-tap reflect
    conv over h is out[q] = sum_d L[d].T-style products with
    d in {-1,0,+1}; L matrices are shift-invariant except the first /
    last block (reflect folds). Stationaries are reused across blocks
    (ordered by d), accumulation in PSUM.
  * blend in the [(h m), w] layout against a host-prepared whT map.
  * expand 16->256 with the zero-masked replicated weight trick,
    per-(hlp,cc,j2) PSUM tiles, drains alternating DVE/Act,
    2 output DMAs per chunk (contiguous 2KB runs in DRAM).
  * ~8 zero matmuls at t~7.5us (gated only by a tiny const DMA) keep
    the PE busy during the input-DMA head so it ramps to full p-state
    before real work arrives.

Sharding: data-parallel over batch, 1 batch per NeuronCore (B=8).
"""

import math

import numpy as np

import concourse.bass as bass
import concourse.tile as tile
from concourse import mybir
import bass_rust
from concourse.bass_utils import run_bass_kernel_spmd

B, C, H, W, MID = 8, 256, 128, 128, 16
K, PAD = 7, 3
F32 = mybir.dt.float32
BF16 = mybir.dt.bfloat16

NCH = 32  # output rows per chunk
NCHUNK = H // NCH
NB = H // 8  # 16 8-row blocks


# ----------------------------------------------------------------- host consts
def _refl(t, n):
    if t < 0:
        return -t
    if t > n - 1:
        return 2 * (n - 1) - t
    return t


def _banded_reflect(g, n):
    """T[out, in]: out[o] = sum_t g[t] * x[refl(o + t - PAD)]."""
    T = np.zeros((n, n), dtype=np.float64)
    for o in range(n):
        for t in range(K):
            T[o, _refl(o + t - PAD, n)] += g[t]
    return T


def _host_consts():
    ax = np.linspace(-(K // 2), K // 2, K, dtype=np.float64)
    e_w = np.exp(-(ax**2) / (2 * 2.5**2))  # wide gaussian (sigma_h)
    e_n = np.exp(-(ax**2) / (2 * 1.0**2))  # narrow gaussian (sigma_v)
    s_h = float(np.outer(e_w, e_n).sum()) + 1e-8
    s_v = float(np.outer(e_n, e_w).sum()) + 1e-8
    gh = [e_w, e_n]  # h-axis taps for paths A, B (unnormalized)
    gw = [e_n / s_h, e_w / s_v]  # w-axis taps carry the normalization

    Th = [_banded_reflect(g, H) for g in gh]  # [h_out, h_in] per path
    TwT = [np.ascontiguousarray(_banded_reflect(g, W).T) for g in gw]

    # block-tridiagonal factorization of Th into (delta, variant) L mats:
    # L[p=(hl,m), p'=(hl',m')] = Th[8*q_out+hl', 8*q_in+hl] * (m==m')
    # variants: 0:interior d=-1, 1:interior d=0, 2:interior d=+1,
    #           3:d=0 for block 0 (top folds), 4:d=0 for block 15
    Ls = []
    for Tp in Th:
        mats = []
        for (qo, qi) in ((1, 0), (1, 1), (1, 2), (0, 0), (NB - 1, NB - 1)):
            Lm = np.zeros((128, 128), np.float64)
            for hl in range(8):
                for hlp in range(8):
                    v = Tp[8 * qo + hlp, 8 * qi + hl]
                    if v != 0.0:
                        for m in range(MID):
                            Lm[hl * MID + m, hlp * MID + m] = v
            mats.append(Lm)
        Ls.append(mats)
        # verify: assembled block-tridiag reproduces Th exactly
        full = np.zeros((H, H))
        for qo in range(NB):
            for qi in range(NB):
                d = qi - qo
                if abs(d) > 1:
                    assert np.allclose(Tp[8*qo:8*qo+8, 8*qi:8*qi+8], 0)
                    continue
                if d == 0:
                    Lm = mats[3] if qo == 0 else (mats[4] if qo == NB - 1 else mats[1])
                else:
                    Lm = mats[d + 1]
                blk = np.zeros((8, 8))
                for hl in range(8):
                    for hlp in range(8):
                        blk[hlp, hl] = Lm[hl * MID, hlp * MID]
                full[8*qo:8*qo+8, 8*qi:8*qi+8] = blk
        assert np.allclose(full, Tp), "block-tridiag mismatch"
    return gh, Th, TwT, Ls


GH, TH, TWT, LS = _host_consts()

# const layouts
NC_EARLY = 32 + 512  # wrT halves [128,16]x2 + zero row region

OFF_TWTA = 0
OFF_TWTB = 128
OFF_L = 256                        # 3 "paths" (B, A, -B) x 5 variants x 128
OFF_WET = OFF_L + 15 * 128         # 2176
OFF_WHT = OFF_WET + 4 * C          # 3200
NC_LATE = OFF_WHT + H * MID        # 5248


def _build_const_early(w_reduce):
    ce = np.zeros((128, NC_EARLY), dtype=np.float64)
    wrT = w_reduce.T.astype(np.float64)  # [C, MID]
    ce[:, 0:MID] = wrT[0:128]
    ce[:, MID:2 * MID] = wrT[128:256]
    return ce


def _build_const_late(w_expand):
    cl = np.zeros((128, NC_LATE), dtype=np.float64)
    cl[:, OFF_TWTA:OFF_TWTA + 128] = TWT[0]
    cl[:, OFF_TWTB:OFF_TWTB + 128] = TWT[1]
    # L sets: 0 -> path B, 1 -> path A, 2 -> negated path B (for the
    # PE-computed difference path D = A - B)
    for p, mats in enumerate((LS[1], LS[0], [-m for m in LS[1]])):
        for v in range(5):
            o = OFF_L + (p * 5 + v) * 128
            cl[:, o:o + 128] = mats[v]
    weT = w_expand.T.astype(np.float64)  # [MID, C]
    wet = np.zeros((128, 4 * C), np.float64)
    for p in range(128):
        v = (p // 16) % 4
        wet[p, v * C:(v + 1) * C] = weT[p % 16]
    cl[:, OFF_WET:OFF_WET + 4 * C] = wet
    return cl


def _whT(angle_map_b):
    wh = np.cos(np.asarray(angle_map_b, np.float64)) ** 2  # [H, W]
    hl = np.arange(128) // MID
    kb = np.arange(NB)
    return wh[(8 * kb[None, :] + hl[:, None])].reshape(128, H * MID)


# ----------------------------------------------------------------- bass module
def build_nc(split_multiwaits=True):
    nc = bass.Bass()

    x_in = nc.dram_tensor("x", [C, H, W], BF16, kind="ExternalInput")
    ce_in = nc.dram_tensor("c_early", [128, NC_EARLY], BF16, kind="ExternalInput")
    cl_in = nc.dram_tensor("c_late", [128, NC_LATE], BF16, kind="ExternalInput")
    out_dram = nc.dram_tensor("out", [C, H, W], BF16, kind="ExternalOutput")

    from contextlib import ExitStack

    with tile.TileContext(nc) as tc, ExitStack() as es:
        consts = es.enter_context(tc.tile_pool(name="consts", bufs=1))
        xpool = es.enter_context(tc.tile_pool(name="xpool", bufs=1))
        x3pool = es.enter_context(tc.tile_pool(name="x3", bufs=1))
        uwpool = es.enter_context(tc.tile_pool(name="uw", bufs=1))
        zpool = es.enter_context(tc.tile_pool(name="z", bufs=2))
        olppool = es.enter_context(tc.tile_pool(name="olp", bufs=3))
        opool = es.enter_context(tc.tile_pool(name="ostage", bufs=2))
        ps1pool = es.enter_context(tc.tile_pool(name="ps1", bufs=1, space="PSUM"))
        pswpool = es.enter_context(tc.tile_pool(name="psw", bufs=1, space="PSUM"))
        # psh (H-conv out) and pso (expand out) share one rotating pool of
        # 2-bank tiles: more runway between a matmul and the drain that
        # frees its slot, so the PE rarely stalls on slot rotation
        psopool = es.enter_context(tc.tile_pool(name="pso", bufs=3, space="PSUM"))
        pshpool = psopool

        cearly = consts.tile([128, NC_EARLY], BF16)
        nc.sync.dma_start(out=cearly, in_=ce_in[:])

        xt = [[None, None] for _ in range(8)]

        def dma_x(g):
            for ch in range(2):
                t = xpool.tile([128, 16, W], BF16, tag=f"x{g}_{ch}")
                nc.sync.dma_start(
                    out=t, in_=x_in[ch * 128:(ch + 1) * 128, g * 16:(g + 1) * 16, :]
                )
                xt[g][ch] = t

        # issue order tuned to expected consumption times: conv consts
        # (TwT + L) after the first two x groups, the fat expand/blend
        # consts (wet + whT) before the first blend needs them
        clate = consts.tile([128, NC_LATE], BF16)
        for g in range(2):
            dma_x(g)
        nc.sync.dma_start(out=clate[:, 0:OFF_WET], in_=cl_in[:, 0:OFF_WET])
        for g in range(2, 4):
            dma_x(g)
        nc.sync.dma_start(out=clate[:, OFF_WET:], in_=cl_in[:, OFF_WET:])
        for g in range(4, 8):
            dma_x(g)

        WR = [cearly[:, 0:MID], cearly[:, MID:2 * MID]]
        ZROW = cearly[0:1, 32:32 + 512]
        ZCOL = cearly[0:1, 32:32 + 128]
        TWTv = [clate[:, OFF_TWTA:OFF_TWTA + 128],
                clate[:, OFF_TWTB:OFF_TWTB + 128]]
        Lv = [
            [clate[:, OFF_L + (p * 5 + v) * 128:OFF_L + (p * 5 + v + 1) * 128]
             for v in range(5)]
            for p in range(3)  # 0: B, 1: A, 2: -B
        ]
        WETv = clate[:, OFF_WET:OFF_WET + 4 * C]
        WHTv = clate[:, OFF_WHT:OFF_WHT + H * MID].rearrange(
            "p (kb w) -> p kb w", kb=NB
        )

        X3 = x3pool.tile([128, H * MID], BF16)  # [w, (h m)]
        Uw = [uwpool.tile([128, NB, W], BF16, tag=f"uw{p}", name=f"uw{p}")
              for p in range(2)]  # [(hl m), (blk, w')] per path

        ps1s, pshs, olps = {}, {}, {}

        def emit_warmup(n):
            """dummy matmuls into the ps1 slot: ramp PE during DMA head."""
            for i in range(n):
                ps1 = ps1pool.tile([128, 512], F32, tag="ps1")
                nc.tensor.matmul(
                    ps1, lhsT=ZCOL, rhs=ZROW,
                    start=True, stop=True, skip_group_check=True,
                )

        def emit_s1(k):
            """x_low for 32 h rows -> psum [w, (hl m)]."""
            ps1 = ps1pool.tile([128, 512], F32, tag="ps1")
            ps1s[k] = ps1
            for hl in range(32):
                h = NCH * k + hl
                fo = hl * MID
                nc.tensor.matmul(
                    ps1[:, fo:fo + MID], lhsT=xt[h // 16][0][:, h % 16, :],
                    rhs=WR[0], start=True, stop=False,
                )
                nc.tensor.matmul(
                    ps1[:, fo:fo + MID], lhsT=xt[h // 16][1][:, h % 16, :],
                    rhs=WR[1], start=False, stop=True,
                )

        def emit_s1drain(k):
            eng = nc.vector.tensor_copy if k % 2 == 0 else nc.scalar.copy
            kw = {"out": X3[:, NCH * MID * k:NCH * MID * (k + 1)], "in_": ps1s[k]}
            eng(**kw)

        def emit_wt(k):
            """psW[(hl m), w'] = X3_block.T @ TwT; drain to Uw. Paths
            run sequentially through a single 1-bank psum tile (the PE
            has slack in the input phase; this frees a bank for pso)."""
            for p in range(2):
                psw = pswpool.tile([128, 4, W], F32, tag="psw")
                for kb in range(4):
                    qb = 4 * k + kb
                    nc.tensor.matmul(
                        psw[:, kb],
                        lhsT=X3[:, qb * 128:(qb + 1) * 128],
                        rhs=TWTv[p], start=True, stop=True,
                    )
                if p == 0:
                    nc.vector.tensor_copy(
                        out=Uw[0][:, 4 * k:4 * k + 4, :], in_=psw
                    )
                else:
                    nc.scalar.copy(out=Uw[1][:, 4 * k:4 * k + 4, :], in_=psw)

        def emit_ht(k):
            """block-tridiagonal H-conv: psH[q] = sum_d L[d] blocks."""
            psh = pshpool.tile([128, 2, 4, W], F32, tag="pso", name=f"psh{k}")
            pshs[k] = psh
            # slot 0 accumulates path B; slot 1 accumulates the
            # difference path D = A - B (extra negated-L taps), so the
            # blend needs only 2 DVE ops and no gpsimd on the critical
            # path. Wide rhs applies the same L block-diagonally; each
            # psum region's accumulation group stays consecutive.
            passes = [(0, [(0, 1)]), (1, [(1, 0), (2, 1)])]
            for slot, sp in passes:
                mms = []
                if k == 0:
                    for s, pu in sp:
                        mms += [
                            (Lv[s][3], Uw[pu][:, 0, :]),
                            (Lv[s][2], Uw[pu][:, 1, :]),
                        ]
                    for i, (L, rhs) in enumerate(mms):
                        nc.tensor.matmul(
                            psh[:, slot, 0], lhsT=L, rhs=rhs,
                            start=(i == 0), stop=(i == len(mms) - 1),
                        )
                    mms = []
                    for s, pu in sp:
                        mms += [
                            (Lv[s][0], Uw[pu][:, 0:3, :]),
                            (Lv[s][1], Uw[pu][:, 1:4, :]),
                            (Lv[s][2], Uw[pu][:, 2:5, :]),
                        ]
                    for i, (L, rhs) in enumerate(mms):
                        nc.tensor.matmul(
                            psh[:, slot, 1:4], lhsT=L, rhs=rhs,
                            start=(i == 0), stop=(i == len(mms) - 1),
                        )
                elif k == NCHUNK - 1:
                    mms = [(Lv[s][v], Uw[pu][:, 11 + v:14 + v, :])
                           for s, pu in sp for v in range(3)]
                    for i, (L, rhs) in enumerate(mms):
                        nc.tensor.matmul(
                            psh[:, slot, 0:3], lhsT=L, rhs=rhs,
                            start=(i == 0), stop=(i == len(mms) - 1),
                        )
                    mms = []
                    for s, pu in sp:
                        mms += [
                            (Lv[s][0], Uw[pu][:, 14, :]),
                            (Lv[s][4], Uw[pu][:, 15, :]),
                        ]
                    for i, (L, rhs) in enumerate(mms):
                        nc.tensor.matmul(
                            psh[:, slot, 3], lhsT=L, rhs=rhs,
                            start=(i == 0), stop=(i == len(mms) - 1),
                        )
                else:
                    q0 = 4 * k
                    mms = []
                    for s, pu in sp:
                        mms += [
                            (Lv[s][0], Uw[pu][:, q0 - 1:q0 + 3, :]),
                            (Lv[s][1], Uw[pu][:, q0:q0 + 4, :]),
                            (Lv[s][2], Uw[pu][:, q0 + 1:q0 + 5, :]),
                        ]
                    for i, (L, rhs) in enumerate(mms):
                        nc.tensor.matmul(
                            psh[:, slot], lhsT=L, rhs=rhs,
                            start=(i == 0), stop=(i == len(mms) - 1),
                        )

        def emit_blend(k):
            """OLp = B + whT*D in [(hl m), (kb w)] layout; D = A - B was
            already formed on the PE, so this is 2 DVE ops (each with
            one PSUM operand) and nothing else on the critical path."""
            psh = pshs[k]
            p = zpool.tile([128, 4, W], BF16, tag="p")
            nc.vector.tensor_mul(
                out=p, in0=psh[:, 1], in1=WHTv[:, 4 * k:4 * k + 4, :]
            )
            OLp = olppool.tile([128, 4 * W], BF16, tag="olp")
            OLr = OLp.rearrange("p (kb w) -> p kb w", kb=4)
            nc.vector.tensor_add(out=OLr, in0=p, in1=psh[:, 0])
            olps[k] = OLp

        def emit_filler(n):
            """zero matmuls (no data deps beyond cearly): keep the PE
            busy through drain-gated or DMA-gated stretches so it holds
            full p-state."""
            for _ in range(n):
                ps1 = ps1pool.tile([128, 512], F32, tag="ps1")
                nc.tensor.matmul(
                    ps1, lhsT=ZCOL, rhs=ZROW,
                    start=True, stop=True, skip_group_check=True,
                )

        def emit_expand(k, hlps, osts, weave=None):
            """expand 16->256 for a subset of hlp values (+ drains); the
            optional `weave` callback emits other PE work between hlp
            pieces so pso-slot waits never idle the PE. j2 pairs share a
            2-bank pso tile drained by one 1024-col copy."""
            OLp = olps[k]
            for hlp in hlps:
                for cc in range(2):
                    pso = psopool.tile([128, 2, 4, W], F32, tag="pso")
                    for j2 in range(2):
                        hl = hlp * 2 + j2
                        b = 64 * (hl // 4)
                        v = hl % 4
                        nc.tensor.matmul(
                            pso[:, j2],
                            lhsT=WETv[b:b + 64,
                                      (v * 2 + cc) * 128:(v * 2 + cc + 1) * 128],
                            rhs=OLp[b:b + 64, :],
                            start=True, stop=True,
                            tile_position=(b, 0),
                        )
                    # both engines drain one j2 half each, in parallel
                    nc.vector.tensor_copy(
                        out=osts[cc][:, :, hlp * 2, :], in_=pso[:, 0]
                    )
                    nc.scalar.copy(
                        out=osts[cc][:, :, hlp * 2 + 1, :], in_=pso[:, 1]
                    )
                if weave is not None:
                    weave(hlp)

        def emit_ost(k):
            osts = []
            for cc in range(2):
                ost = opool.tile([128, 4, 8, W], BF16, tag=f"ost{cc}")
                osts.append(ost)
            return osts

        def emit_outdma(k, osts):
            for cc in range(2):
                o_r = out_dram[cc * 128:(cc + 1) * 128].rearrange(
                    "c (kb hl) w -> c kb hl w", hl=8
                )
                nc.sync.dma_start(
                    out=o_r[:, 4 * k:4 * k + 4, :, :], in_=osts[cc]
                )

        # software pipeline, ordered by expected operand readiness so the
        # in-order PE queue never head-blocks; expand matmuls are woven
        # with the next chunk's conv matmuls (or fillers) so pso-slot
        # drain waits never idle the PE and it holds full p-state
        # ready-time-driven: chunk 0's full path races ahead of the
        # input stream; the drain-paced expand stretches are woven at
        # per-hlp granularity with later chunks' s1/wt/ht matmuls so the
        # PE keeps streaming (and its p-state) while drains catch up
        emit_warmup(4)
        emit_s1(0)
        emit_s1drain(0)
        emit_wt(0)
        emit_s1(1)
        emit_s1drain(1)
        emit_wt(1)
        emit_ht(0)
        emit_blend(0)
        emit_s1(2)
        emit_s1drain(2)
        ost0 = emit_ost(0)

        def w0(h):
            if h == 0:
                emit_wt(2)
            elif h == 1:
                emit_s1(3)
                emit_s1drain(3)
            elif h == 2:
                emit_ht(1)
        emit_expand(0, range(4), ost0, weave=w0)
        emit_outdma(0, ost0)
        emit_blend(1)
        ost1 = emit_ost(1)

        def w1(h):
            if h == 0:
                emit_wt(3)
            elif h == 2:
                emit_ht(2)
        emit_expand(1, range(4), ost1, weave=w1)
        emit_outdma(1, ost1)
        emit_blend(2)
        ost2 = emit_ost(2)
        emit_expand(2, range(4), ost2,
                    weave=lambda h: emit_ht(3) if h == 1 else None)
        emit_outdma(2, ost2)
        emit_blend(3)
        ost3 = emit_ost(3)
        emit_expand(3, range(4), ost3)
        emit_outdma(3, ost3)

    if split_multiwaits:
        _split_multiwaits(nc)
    return nc


def _split_multiwaits(nc):
    """Walrus in this toolchain accepts at most one sync-wait per
    instruction; hoist extras onto same-engine nops just before it."""
    n_new = 0
    for f in nc.m.functions:
        for bb in f.blocks:
            out, changed = [], False
            for ins in bb.instructions:
                si = ins.sync_info
                if si is not None and len(si.on_wait) > 1:
                    changed = True
                    waits = list(si.on_wait)
                    for w in waits[:-1]:
                        n_new += 1
                        nop = bass_rust.InstNoOp(
                            name=f"I-mwsplit-{n_new}", engine=ins.engine
                        )
                        nop.sync_info = mybir.SyncInfo(on_wait=[w], on_update=[])
                        out.append(nop)
                    ins.sync_info = mybir.SyncInfo(
                        on_wait=[waits[-1]], on_update=list(si.on_update)
                    )
                out.append(ins)
            if changed:
                bb.instructions = out
    return n_new


_NC = None


def _get_nc():
    global _NC
    if _NC is None:
        _NC = build_nc()
    return _NC


def make_in_maps(x, angle_map, w_reduce, w_expand):
    bf = mybir.dt.np(BF16)
    ce = _build_const_early(np.asarray(w_reduce, np.float64)).astype(bf)
    cl_shared = _build_const_late(np.asarray(w_expand, np.float64))
    maps = []
    for i in range(B):
        cl = cl_shared.copy()
        cl[:, OFF_WHT:OFF_WHT + H * MID] = _whT(angle_map[i])
        maps.append(
            {
                "x": np.ascontiguousarray(np.asarray(x[i])).astype(bf),
                "c_early": ce,
                "c_late": cl.astype(bf),
            }
        )
    return maps


def kernel(x, angle_map, w_reduce, w_expand):
    nc = _get_nc()
    in_maps = make_in_maps(x, angle_map, w_reduce, w_expand)
    res = run_bass_kernel_spmd(nc, in_maps, core_ids=list(range(B)))
    return np.stack([r["out"] for r in res.results]).astype(np.float32)
